# revision 17
# baseline (speedup 1.0000x reference)
"""Trainium2 Bass kernel for nn_DecoderRNN (LSTM decoder + vocab projection).

Strategy (8 NeuronCores, SPMD):
  - Recurrence tensor-parallel: core v owns h-dims [128v, 128(v+1)). Each step it
    computes its 4x128 gate rows (transposed layout [gate_dim, batch]) with bf16
    matmuls in one PSUM bank, applies fused sigmoid(i,f,o)/tanh(g) on ScalarE,
    updates c/h on VectorE (h lands directly in hall slot 0), then pushes its
    h^T shard [128,64] bf16 SBUF->SBUF to all 7 peers with single-dest
    remote_dma_broadcast (slot j on receiver v holds rank v^j's shard; the XOR
    dest mapping is absorbed by per-core K-chunk permutation of W_hh/W_fc on
    the host). Receivers gate their next-step matmuls on a monotonic remote
    semaphore (+2 per sender-step); the recurrence itself bounds sender skew
    to one step so cumulative waits are race-free.
  - A 1-byte AllGather at kernel start is the rendezvous barrier: remote sems
    are cleared before it, and no rank can send until every rank has cleared.
  - fc projection vocab-parallel: core v owns 6400 vocab rows (V padded to
    51200). W_fc^T (bf16) is SBUF-resident, host-laid-out so each partition
    line is contiguous (128 big DMA descriptors instead of 13k small ones),
    split in 16 (j, half) chunks so early fc tiles only wait on their chunk.
    Steps are batched in pairs (stationary hall [128k,128m]); W_fc^T streams
    as the moving operand in N=512 tiles; b_fc is added during the PSUM drain
    on VectorE which writes bf16 directly (host upcasts to fp32).
  - x_proj = features @ W_ih^T is step-invariant: computed once on device and
    injected into each step's PSUM gate bank via a bf16 identity matmul.
  - fc work for pair p is emitted in slots 2p+2 / 2p+3 as PE filler so the PE
    never idles long enough to re-throttle (HAM) while h broadcasts fly.
"""

import numpy as np
import ml_dtypes

import concourse.bass as bass
import concourse.bacc as bacc
import concourse.tile as tile
import concourse.mybir as mybir
from concourse import bass_utils
from concourse.bass_interp import get_hw_module

BF = ml_dtypes.bfloat16
FP32 = mybir.dt.float32
BF16 = mybir.dt.bfloat16
U8 = mybir.dt.uint8
AF = mybir.ActivationFunctionType

R = 8            # cores
B = 64           # batch
E = 512          # embed
H = 1024         # hidden
V = 50257        # vocab
VP = 51200       # padded vocab (multiple of 8*128)
VS = VP // R     # per-core vocab shard = 6400
NKH = H // 128   # 8 k-chunks over hidden
NKE = E // 128   # 4 k-chunks over embed
FC_N = 512       # fc free-dim tile
NFC = (VS + FC_N - 1) // FC_N  # 13 n-tiles

P2P_INC = 2      # remote_sem inc per single-dest broadcast (16 // 8 dests)
STEP_INC = 7 * P2P_INC  # remote_sem advance per completed step (7 peers)


def _emit(nc, tc, T, wfcT, whhT, wihT, xT, bfc, bias, ident, bar_in, bar_out, out, dbg=None):
    """Returns (rsem, lsem, anchors): anchors are (inst_name, rsem_target)
    pairs for PE waits that must be inserted AFTER Tile scheduling — the
    no-exec scheduling sim cannot model remote semaphore increments and
    would report a deadlock if the waits were visible to it."""
    NP = (T + 1) // 2  # step pairs
    anchors = []

    # ---- p2p semaphores + rendezvous barrier ----
    # Clears must precede this rank's barrier entry: a peer only sends after
    # the AllGather completes, which is after every rank (incl. us) entered,
    # so increments can never race our clear.
    rsem = nc.alloc_semaphore("p2p_remote")
    lsem = nc.alloc_semaphore("p2p_local")
    cl_r = nc.gpsimd.sem_clear(rsem)
    cl_l = nc.gpsimd.sem_clear(lsem)
    cc = nc.gpsimd.collective_compute(
        "AllGather",
        mybir.AluOpType.bypass,
        ins=[bar_in.opt()],
        outs=[bar_out.opt()],
        replica_groups=[list(range(R))],
    )
    tile.add_dep_helper(cc.ins, cl_r.ins, sync=False, reason="clear before barrier entry")
    tile.add_dep_helper(cc.ins, cl_l.ins, sync=False, reason="clear before barrier entry")

    with (
        tc.tile_pool(name="wpool", bufs=1) as wpool,
        tc.tile_pool(name="hpool", bufs=1) as hpool,
        tc.tile_pool(name="spool", bufs=3) as spool,
        tc.tile_pool(name="stage", bufs=6) as stage,
        tc.tile_pool(name="pg", bufs=1, space="PSUM") as pg_pool,
        tc.tile_pool(name="pf", bufs=4, space="PSUM") as pf_pool,
    ):
        # ---- resident weights / constants (small DMAs first) ----
        wih_sb = wpool.tile([128, NKE, 512], BF16, name="wih_sb")
        nc.sync.dma_start(wih_sb[:], wihT)
        x_sb = wpool.tile([128, NKE, B], BF16, name="x_sb")
        nc.sync.dma_start(x_sb[:], xT)
        whh_sb = wpool.tile([128, NKH, 512], BF16, name="whh_sb")
        nc.sync.dma_start(whh_sb[:], whhT)
        bias_sb = wpool.tile([128, 4], FP32, name="bias_sb")
        nc.sync.dma_start(bias_sb[:], bias)
        id_sb = wpool.tile([128, 128], BF16, name="id_sb")
        nc.sync.dma_start(id_sb[:], ident)
        bfc_row = wpool.tile([1, VS], BF16, name="bfc_row")
        nc.sync.dma_start(bfc_row[:], bfc)
        ones_sb = wpool.tile([1, 128], BF16, name="ones_sb")
        nc.vector.memset(ones_sb[:], 1.0)
        # big W_fc^T load: host pre-interleaved so each partition line is
        # contiguous; 16 (j, half) chunks so early fc tiles wait only on
        # their chunk. SWDGE from ScalarE keeps the Sync queue free.
        wfc_sb = wpool.tile([128, NKH, VS], BF16, name="wfc_sb")
        HALF = VS // 2
        for half in range(2):
            for j in range(NKH):
                nc.scalar.dma_start(
                    wfc_sb[:, j, half * HALF : (half + 1) * HALF],
                    wfcT[:, j, half * HALF : (half + 1) * HALF],
                )
        bfc_sb = wpool.tile([128, VS], FP32, name="bfc_sb")

        # ---- x_proj^T + gate biases, [4 gate tiles][128, B], bf16 ----
        xproj_sb = wpool.tile([128, 4, B], BF16, name="xproj_sb")
        for m in range(4):
            px = pg_pool.tile([128, B], FP32, name="px", tag="pg")
            for k in range(NKE):
                nc.tensor.matmul(
                    px[:], wih_sb[:, k, 128 * m : 128 * (m + 1)], x_sb[:, k, :],
                    start=(k == 0), stop=(k == NKE - 1),
                )
            nc.scalar.add(xproj_sb[:, m, :], px[:], bias_sb[:, m : m + 1])

        # persistent state: hall slot j of pair p holds rank (v XOR j)'s
        # h^T shard for the two steps of pair p
        hall = hpool.tile([128, NP, NKH, 2 * B], BF16, name="hall")
        cT = hpool.tile([128, B], FP32, name="cT")
        nc.vector.memset(cT[:], 0.0)

        def fc_emit(p, n_lo, n_hi, msz):
            first_mm, last_mm = None, None
            for n in range(n_lo, n_hi):
                nsz = min(FC_N, VS - n * FC_N)
                pf = pf_pool.tile([128, FC_N], FP32, name="pf", tag="pf")
                for k in range(NKH):
                    i = nc.tensor.matmul(
                        pf[:msz, :nsz],
                        hall[:, p, k, :msz],
                        wfc_sb[:, k, n * FC_N : n * FC_N + nsz],
                        start=(k == 0), stop=(k == NKH - 1),
                    )
                    if first_mm is None:
                        first_mm = i
                    last_mm = i
                st = stage.tile([128, FC_N], BF16, name="st")
                nc.vector.tensor_add(
                    st[:msz, :nsz], pf[:msz, :nsz],
                    bfc_sb[:msz, n * FC_N : n * FC_N + nsz],
                )
                nc.sync.dma_start(
                    out[p, :msz, n * FC_N : n * FC_N + nsz], st[:msz, :nsz]
                )
            return first_mm, last_mm

        FC_SPLIT = 7  # n-tiles in the first half-chunk of a pair

        prev_filler_last = None
        for t in range(T):
            p, s = divmod(t, 2)
            # ---- gates^T in one PSUM bank [128, 4, B], order (i,f,o,g) ----
            pg = pg_pool.tile([128, 4, B], FP32, name="pg", tag="pg")
            first_gates, last_gates = None, None
            for m in range(4):
                i = nc.tensor.matmul(
                    pg[:, m, :], id_sb[:], xproj_sb[:, m, :],
                    start=True, stop=(t == 0),
                )
                if first_gates is None:
                    first_gates = i
                last_gates = i
                if t > 0:
                    pp, ss = divmod(t - 1, 2)
                    for k in range(NKH):
                        last_gates = nc.tensor.matmul(
                            pg[:, m, :],
                            whh_sb[:, k, 128 * m : 128 * (m + 1)],
                            hall[:, pp, k, B * ss : B * (ss + 1)],
                            start=False, stop=(k == NKH - 1),
                        )
            if t > 0:
                anchors.append((first_gates.ins.name, STEP_INC * t, "tensor"))
            # keep PE stream interleaved: this slot's gates run after the
            # previous slot's PE filler work
            if prev_filler_last is not None:
                tile.add_dep_helper(
                    first_gates.ins, prev_filler_last.ins, sync=False,
                    reason="slot order: gates after previous slot's fc filler",
                )
            # ---- activations + c/h update; h lands in hall slot 0 (self) ----
            sio = spool.tile([128, 3, B], FP32, name="sio")
            g_t = spool.tile([128, B], FP32, name="g_t")
            t1 = spool.tile([128, B], FP32, name="t1")
            t2 = spool.tile([128, B], FP32, name="t2")
            tc_t = spool.tile([128, B], FP32, name="tc_t")
            h_self = hall[:, p, 0, B * s : B * (s + 1)]
            with tc.high_priority():
                nc.scalar.activation(sio[:], pg[:, 0:3, :], AF.Sigmoid)
                nc.scalar.activation(g_t[:], pg[:, 3, :], AF.Tanh)
                nc.vector.tensor_mul(t2[:], sio[:, 1, :], cT[:])
                nc.vector.tensor_mul(t1[:], sio[:, 0, :], g_t[:])
                nc.vector.tensor_add(cT[:], t1[:], t2[:])
                nc.scalar.activation(tc_t[:], cT[:], AF.Tanh)
                nc.vector.tensor_mul(h_self, sio[:, 2, :], tc_t[:])
            # ---- p2p broadcast of h shard to the 7 peers ----
            for j in range(1, NKH):
                nc.gpsimd.remote_dma_broadcast(
                    hall[:, p, j, B * s : B * (s + 1)],
                    h_self,
                    remote_sem=rsem,
                    local_sem=lsem,
                    rdests=[(0, j) if k == j else None for k in range(NKH)],
                    queue_num=1,
                )
            trig = nc.gpsimd.trigger_dma(count=None, queue_num=1)
            if t == 0:
                # no rank may send until every rank has cleared its p2p sems;
                # later triggers are gated transitively by the inserted PE
                # waits on peer data
                tile.add_dep_helper(
                    trig.ins, cc.ins, sync=True,
                    reason="first p2p send after rendezvous barrier",
                )
            # ---- PE filler while the broadcasts fly ----
            first_fill, last_fill = None, None
            if t == 0:
                # b_fc broadcast via K=1 ones-matmuls fills slot 0
                for n in range(NFC):
                    nsz = min(FC_N, VS - n * FC_N)
                    pb = pf_pool.tile([128, FC_N], FP32, name="pb", tag="pf")
                    i = nc.tensor.matmul(
                        pb[:, :nsz], ones_sb[:], bfc_row[:, n * FC_N : n * FC_N + nsz],
                        start=True, stop=True,
                    )
                    if first_fill is None:
                        first_fill = i
                    last_fill = i
                    nc.vector.tensor_copy(
                        bfc_sb[:, n * FC_N : n * FC_N + nsz], pb[:, :nsz]
                    )
            elif t >= 2:
                q, half = divmod(t - 2, 2)
                if half == 0:
                    first_fill, last_fill = fc_emit(q, 0, FC_SPLIT, 2 * B)
                else:
                    first_fill, last_fill = fc_emit(q, FC_SPLIT, NFC, 2 * B)
            if first_fill is not None:
                tile.add_dep_helper(
                    first_fill.ins, last_gates.ins, sync=False,
                    reason="slot order: fc filler after this slot's gates",
                )
                prev_filler_last = last_fill
            else:
                prev_filler_last = last_gates
        # ---- tail: last pair not covered in-loop ----
        fm, _ = fc_emit(NP - 1, 0, NFC, B if (T % 2) else 2 * B)
        anchors.append((fm.ins.name, STEP_INC * T, "tensor"))
        if dbg is not None:
            dd = nc.sync.dma_start(dbg, hall[:])
            anchors.append((dd.ins.name, STEP_INC * T, "sync"))
    return rsem, lsem, anchors


def build(T, with_dbg=False):
    nc = bacc.Bacc(
        "TRN2",
        target_bir_lowering=False,
        debug=False,
        enable_asserts=False,
        num_devices=R,
        num_swdge_queues=2,
    )
    NP = (T + 1) // 2
    wfcT = nc.dram_tensor("wfcT", [128, NKH, VS], BF16, kind="ExternalInput").ap()
    whhT = nc.dram_tensor("whhT", [128, NKH, 512], BF16, kind="ExternalInput").ap()
    wihT = nc.dram_tensor("wihT", [128, NKE, 512], BF16, kind="ExternalInput").ap()
    xT = nc.dram_tensor("xT", [128, NKE, B], BF16, kind="ExternalInput").ap()
    bfc = nc.dram_tensor("bfc", [1, VS], BF16, kind="ExternalInput").ap()
    bias = nc.dram_tensor("bias", [128, 4], FP32, kind="ExternalInput").ap()
    ident = nc.dram_tensor("ident", [128, 128], BF16, kind="ExternalInput").ap()
    bar_in = nc.dram_tensor("bar_in", [1, 1], U8, kind="Internal").ap()
    bar_out = nc.dram_tensor("bar_out", [R, 1], U8, kind="Internal").ap()
    out = nc.dram_tensor("out", [NP, 128, VS], BF16, kind="ExternalOutput").ap()
    dbg = (
        nc.dram_tensor("dbg", [128, NP, NKH, 2 * B], BF16, kind="ExternalOutput").ap()
        if with_dbg
        else None
    )

    with tile.TileContext(nc) as tc:
        rsem, lsem, anchors = _emit(
            nc, tc, T, wfcT, whhT, wihT, xT, bfc, bias, ident, bar_in, bar_out, out, dbg
        )
    # Post-Tile: insert the PE waits on peer h-shard arrival in front of
    # their anchor matmuls, and drain our own sends before kernel end.
    # (Same post-schedule insertion pattern as insert_bir_kernel_barrier_sem_inc.)
    def _find(name):
        for b in nc.main_func.blocks:
            for i, inst in enumerate(b.instructions):
                if inst.name == name:
                    return b, i
        raise KeyError(name)

    for name, target, eng in anchors:
        w = getattr(nc, eng).wait_ge(rsem, target)
        wb, wi_idx = _find(w.ins.name)
        wi = wb.instructions[wi_idx]
        del wb.instructions[wi_idx]
        ab, a_idx = _find(name)
        ab.instructions.insert(a_idx, wi)
    nc.gpsimd.wait_ge(lsem, 16 * (NKH - 1) * T)
    nc.compile()
    nc.m = get_hw_module(nc.m)
    return nc


_NC_CACHE = {}


def get_nc(T):
    if T not in _NC_CACHE:
        _NC_CACHE[T] = build(T)
    return _NC_CACHE[T]


def _xor_chunks(mat_kT, v):
    """[1024, N] h-dim-major -> [128, 8, N] where chunk j = rows of the rank
    whose broadcast lands in hall slot j on core v. Measured on trn2.8x1:
    slot j <- rank v^j for j<4; the D2D lanes add a fixed ^2 swap, so
    slot j <- rank v^j^2 for j>=4."""
    out = np.empty((128, NKH, mat_kT.shape[1]), mat_kT.dtype)
    for j in range(NKH):
        r = v ^ (j if j < 4 else j ^ 2)
        out[:, j, :] = mat_kT[128 * r : 128 * (r + 1), :]
    return out


def make_in_maps(features, W_ih, W_hh, b_ih, b_hh, W_fc, b_fc):
    features = np.asarray(features, np.float32)
    W_ih = np.asarray(W_ih, np.float32)
    W_hh = np.asarray(W_hh, np.float32)
    W_fc = np.asarray(W_fc, np.float32)
    b = np.asarray(b_ih, np.float32) + np.asarray(b_hh, np.float32)
    b_fc = np.asarray(b_fc, np.float32)

    # xT / wihT: embed-dim-major [512, N] -> [128, NKE, N] (k p interleave)
    xT_np = np.ascontiguousarray(
        features.T.reshape(NKE, 128, B).transpose(1, 0, 2)
    ).astype(BF)
    ident_np = np.eye(128, dtype=np.float32).astype(BF)
    W_fc_pad = np.zeros((VP, H), np.float32)
    W_fc_pad[:V] = W_fc
    bfc_pad = np.zeros((VP,), np.float32)
    bfc_pad[:V] = b_fc

    in_maps = []
    for d in range(R):
        # gate order (i, f, o, g); PyTorch rows are (i, f, g, o)
        gsel = np.concatenate(
            [np.arange(g * H + d * 128, g * H + (d + 1) * 128) for g in (0, 1, 3, 2)]
        )
        whhT_full = np.ascontiguousarray(W_hh[gsel].T)          # [1024, 512]
        wihT_full = np.ascontiguousarray(W_ih[gsel].T)          # [512, 512]
        wfcT_full = np.ascontiguousarray(W_fc_pad[d * VS : (d + 1) * VS].T)  # [1024, VS]
        whhT_np = np.ascontiguousarray(_xor_chunks(whhT_full, d)).astype(BF)
        wfcT_np = np.ascontiguousarray(_xor_chunks(wfcT_full, d)).astype(BF)
        wihT_np = np.ascontiguousarray(
            wihT_full.reshape(NKE, 128, 512).transpose(1, 0, 2)
        ).astype(BF)
        bias_np = np.ascontiguousarray(b[gsel].reshape(4, 128).T)
        bfc_np = bfc_pad[d * VS : (d + 1) * VS].reshape(1, VS).astype(BF)
        in_maps.append(
            {
                "wfcT": wfcT_np,
                "whhT": whhT_np,
                "wihT": wihT_np,
                "xT": xT_np,
                "bfc": bfc_np,
                "bias": bias_np,
                "ident": ident_np,
            }
        )
    return in_maps


def assemble(results, T):
    """results: list of per-core dicts with 'out' [NP, 128, VS] bf16 -> [B, T, V] fp32."""
    NP = (T + 1) // 2
    full = np.concatenate(
        [results[d]["out"].astype(np.float32) for d in range(R)], axis=2
    )  # [NP, 128, VP]
    full = full.reshape(NP, 2, B, VP).transpose(2, 0, 1, 3).reshape(B, 2 * NP, VP)
    return np.ascontiguousarray(full[:, :T, :V])


def kernel(features, W_ih, W_hh, b_ih, b_hh, W_fc, b_fc, max_seq_len):
    T = int(max_seq_len)
    nc = get_nc(T)
    in_maps = make_in_maps(features, W_ih, W_hh, b_ih, b_hh, W_fc, b_fc)
    res = bass_utils.run_bass_kernel_spmd(nc, in_maps, core_ids=list(range(R)))
    return assemble(res.results, T)


# revision 18
# speedup vs baseline: 2.2991x; 2.2991x over previous
"""Trainium2 Bass kernel for nn_DecoderRNN (LSTM decoder + vocab projection).

Strategy (8 NeuronCores, SPMD):
  - Recurrence tensor-parallel: core d owns h-dims [128d, 128(d+1)). Each step it
    computes its 4x128 gate rows (transposed layout [gate_dim, batch]) with bf16
    matmuls accumulating in one PSUM bank (gate order i,f,o,g so one fused
    sigmoid covers i/f/o and one tanh covers g), updates c/h on VectorE, and
    AllGathers the bf16 h^T shard ([128,64] per rank -> [1024,64]) so every
    core has the full hidden state for the next step.
  - fc projection vocab-parallel: core d owns 6400 vocab rows (V padded to
    51200). W_fc^T (bf16) is SBUF-resident and host-pre-interleaved so each
    partition line is contiguous: the 13.1MB load is 16 chunks of 128 big
    descriptors instead of ~13k 1KB ones, cutting startup DMA congestion.
    Steps are batched in pairs so the stationary operand is [128k, 128m]
    (m = 2 steps x 64 batch) at full PE utilization, streaming W_fc^T in
    N=512 tiles. b_fc is added during the PSUM->SBUF drain on VectorE which
    writes bf16 directly (host upcasts; halves the output-store DMA bytes).
  - x_proj = features @ W_ih^T is step-invariant: computed once on device,
    stored bf16, and injected into each step's PSUM bank via a bf16 identity
    matmul.
  - The W_fc^T stream and the b_fc broadcast fill the first AllGather's
    launch-skew window; fc work for pair p is emitted one step late, in half
    chunks, pinned behind each slot's gate matmuls (add_dep_helper) so the PE
    always has filler during the per-step AllGather flight.
"""

import numpy as np
import ml_dtypes

import concourse.bass as bass
import concourse.bacc as bacc
import concourse.tile as tile
import concourse.mybir as mybir
from concourse import bass_utils
from concourse.bass_interp import get_hw_module

BF = ml_dtypes.bfloat16
FP32 = mybir.dt.float32
BF16 = mybir.dt.bfloat16
AF = mybir.ActivationFunctionType

R = 8            # cores
B = 64           # batch
E = 512          # embed
H = 1024         # hidden
V = 50257        # vocab
VP = 51200       # padded vocab (multiple of 8*128)
VS = VP // R     # per-core vocab shard = 6400
NKH = H // 128   # 8 k-chunks over hidden
NKE = E // 128   # 4 k-chunks over embed
FC_N = 512       # fc free-dim tile
NFC = (VS + FC_N - 1) // FC_N  # 13 n-tiles (12x512 + 1x256)


def _emit(nc, tc, T, reps, wfcT, whhT, wihT, xT, bfc, bias, ident, out):
    NP = (T + 1) // 2  # step pairs
    with (
        tc.tile_pool(name="wpool", bufs=1) as wpool,
        tc.tile_pool(name="hpool", bufs=1) as hpool,
        tc.tile_pool(name="spool", bufs=3) as spool,
        tc.tile_pool(name="stage", bufs=6) as stage,
        tc.tile_pool(name="pg", bufs=1, space="PSUM") as pg_pool,
        tc.tile_pool(name="pf", bufs=4, space="PSUM") as pf_pool,
        tc.tile_pool(name="dram", bufs=2, space="DRAM") as dram,
    ):
        # ---- resident weights / constants (small DMAs first) ----
        wih_sb = wpool.tile([128, NKE, 512], BF16, name="wih_sb")
        nc.sync.dma_start(wih_sb[:], wihT)
        x_sb = wpool.tile([128, NKE, B], BF16, name="x_sb")
        nc.sync.dma_start(x_sb[:], xT)
        whh_sb = wpool.tile([128, NKH, 512], BF16, name="whh_sb")
        nc.sync.dma_start(whh_sb[:], whhT)
        bias_sb = wpool.tile([128, 4], FP32, name="bias_sb")
        nc.sync.dma_start(bias_sb[:], bias)
        id_sb = wpool.tile([128, 128], BF16, name="id_sb")
        nc.sync.dma_start(id_sb[:], ident)
        bfc_row = wpool.tile([1, VS], BF16, name="bfc_row")
        nc.sync.dma_start(bfc_row[:], bfc)
        ones_sb = wpool.tile([1, 128], BF16, name="ones_sb")
        nc.vector.memset(ones_sb[:], 1.0)
        # big W_fc^T load: host pre-interleaved, contiguous per partition line
        # -> 128 big descriptors per chunk. Issued inside the early step slots
        # (SWDGE from ScalarE) so the stream doesn't delay the step-critical
        # agi/hall DMAs on the Sync queue.
        wfc_sb = wpool.tile([128, NKH, VS], BF16, name="wfc_sb")
        HALF = VS // 2

        def wfc_load(c_lo, c_hi):
            for c in range(c_lo, c_hi):
                half, j = divmod(c, NKH)
                nc.scalar.dma_start(
                    wfc_sb[:, j, half * HALF : (half + 1) * HALF],
                    wfcT[:, j, half * HALF : (half + 1) * HALF],
                )

        bfc_sb = wpool.tile([128, VS], FP32, name="bfc_sb")

        # ---- x_proj^T + gate biases, [128, 4, B], bf16, order (i,f,o,g) ----
        xproj_sb = wpool.tile([128, 4, B], BF16, name="xproj_sb")
        for m in range(4):
            px = pg_pool.tile([128, B], FP32, name="px", tag="pg")
            for k in range(NKE):
                nc.tensor.matmul(
                    px[:], wih_sb[:, k, 128 * m : 128 * (m + 1)], x_sb[:, k, :],
                    start=(k == 0), stop=(k == NKE - 1),
                )
            nc.scalar.add(xproj_sb[:, m, :], px[:], bias_sb[:, m : m + 1])

        # persistent state
        hall = hpool.tile([128, NP, NKH, 2 * B], BF16, name="hall")
        cT = hpool.tile([128, B], FP32, name="cT")

        def fc_emit(p, n_lo, n_hi, msz):
            first_mm, last_mm = None, None
            for n in range(n_lo, n_hi):
                nsz = min(FC_N, VS - n * FC_N)
                pf = pf_pool.tile([128, FC_N], FP32, name="pf", tag="pf")
                for k in range(NKH):
                    i = nc.tensor.matmul(
                        pf[:msz, :nsz],
                        hall[:, p, k, :msz],
                        wfc_sb[:, k, n * FC_N : n * FC_N + nsz],
                        start=(k == 0), stop=(k == NKH - 1),
                    )
                    if first_mm is None:
                        first_mm = i
                    last_mm = i
                st = stage.tile([128, FC_N], BF16, name="st")
                nc.vector.tensor_add(
                    st[:msz, :nsz], pf[:msz, :nsz],
                    bfc_sb[:msz, n * FC_N : n * FC_N + nsz],
                )
                nc.sync.dma_start(
                    out[p, :msz, n * FC_N : n * FC_N + nsz], st[:msz, :nsz]
                )
            return first_mm, last_mm

        FC_SPLIT = 7  # n-tiles in the first half-chunk of a pair

        for rep in range(reps):
            nc.vector.memset(cT[:], 0.0)
            prev_filler_last = None
            for t in range(T):
                p, s = divmod(t, 2)
                # ---- gates^T in one PSUM bank [128, 4, B] ----
                pg = pg_pool.tile([128, 4, B], FP32, name="pg", tag="pg")
                first_gates, last_gates = None, None
                for m in range(4):
                    i = nc.tensor.matmul(
                        pg[:, m, :], id_sb[:], xproj_sb[:, m, :],
                        start=True, stop=(t == 0),
                    )
                    if first_gates is None:
                        first_gates = i
                    last_gates = i
                    if t > 0:
                        pp, ss = divmod(t - 1, 2)
                        for k in range(NKH):
                            last_gates = nc.tensor.matmul(
                                pg[:, m, :],
                                whh_sb[:, k, 128 * m : 128 * (m + 1)],
                                hall[:, pp, k, B * ss : B * (ss + 1)],
                                start=False, stop=(k == NKH - 1),
                            )
                # keep PE stream interleaved: this slot's gates run after the
                # previous slot's PE filler work
                if prev_filler_last is not None:
                    tile.add_dep_helper(
                        first_gates.ins, prev_filler_last.ins, sync=False,
                        reason="slot order: gates after previous slot's fc filler",
                    )
                # ---- fused activations + c/h update ----
                sio = spool.tile([128, 3, B], FP32, name="sio")
                g_t = spool.tile([128, B], FP32, name="g_t")
                t1 = spool.tile([128, B], FP32, name="t1")
                t2 = spool.tile([128, B], FP32, name="t2")
                tc_t = spool.tile([128, B], FP32, name="tc_t")
                h_bf = spool.tile([128, B], BF16, name="h_bf")
                with tc.high_priority():
                    nc.scalar.activation(sio[:], pg[:, 0:3, :], AF.Sigmoid)
                    nc.scalar.activation(g_t[:], pg[:, 3, :], AF.Tanh)
                    nc.vector.tensor_mul(t2[:], sio[:, 1, :], cT[:])
                    nc.vector.tensor_mul(t1[:], sio[:, 0, :], g_t[:])
                    nc.vector.tensor_add(cT[:], t1[:], t2[:])
                    nc.scalar.activation(tc_t[:], cT[:], AF.Tanh)
                    nc.vector.tensor_mul(h_bf[:], sio[:, 2, :], tc_t[:])
                # ---- AllGather h^T shard -> full h^T ----
                agi = dram.tile([128, B], BF16, name=f"agi{t}", tag=f"agi{t}")
                ago = dram.tile([H, B], BF16, name=f"ago{t}", tag=f"ago{t}")
                ago_pkn = ago.rearrange("(k p) n -> p k n", p=128)
                with tc.high_priority():
                    nc.sync.dma_start(agi[:], h_bf[:])
                    nc.gpsimd.collective_compute(
                        "AllGather",
                        mybir.AluOpType.bypass,
                        ins=[agi.opt()],
                        outs=[ago.opt()],
                        replica_groups=[list(range(R))],
                    )
                    # split so the first gate matmuls can start on the first
                    # (small) piece as early as possible
                    nc.sync.dma_start(
                        hall[:, p, 0:2, B * s : B * (s + 1)], ago_pkn[:, 0:2, :]
                    )
                    nc.sync.dma_start(
                        hall[:, p, 2:8, B * s : B * (s + 1)], ago_pkn[:, 2:8, :]
                    )
                # W_fc^T stream fits inside the first AllGather's launch-skew
                # window
                if rep == 0 and t == 0:
                    wfc_load(0, 2 * NKH)
                # ---- PE filler for this slot's AG flight ----
                first_fill, last_fill = None, None
                if t <= 1 and rep == 0:
                    # b_fc broadcast via K=1 ones-matmuls fills slots 0-1
                    for n in range(7 * t, min(7 * (t + 1), NFC)):
                        nsz = min(FC_N, VS - n * FC_N)
                        pb = pf_pool.tile([128, FC_N], FP32, name="pb", tag="pf")
                        i = nc.tensor.matmul(
                            pb[:, :nsz], ones_sb[:], bfc_row[:, n * FC_N : n * FC_N + nsz],
                            start=True, stop=True,
                        )
                        if first_fill is None:
                            first_fill = i
                        last_fill = i
                        nc.vector.tensor_copy(
                            bfc_sb[:, n * FC_N : n * FC_N + nsz], pb[:, :nsz]
                        )
                elif t >= 2:
                    # fc chunk for a pair whose data landed at step t-1
                    q, half = divmod(t - 2, 2)
                    if half == 0:
                        first_fill, last_fill = fc_emit(q, 0, FC_SPLIT, 2 * B)
                    else:
                        first_fill, last_fill = fc_emit(q, FC_SPLIT, NFC, 2 * B)
                if first_fill is not None:
                    tile.add_dep_helper(
                        first_fill.ins, last_gates.ins, sync=False,
                        reason="slot order: fc filler after this slot's gates",
                    )
                    prev_filler_last = last_fill
                else:
                    prev_filler_last = last_gates
            # ---- tail: last pair(s) not covered in-loop ----
            fc_emit(NP - 1, 0, NFC, B if (T % 2) else 2 * B)


def build(T, reps=1):
    nc = bacc.Bacc(
        "TRN2",
        target_bir_lowering=False,
        debug=False,
        enable_asserts=False,
        num_devices=R,
    )
    NP = (T + 1) // 2
    wfcT = nc.dram_tensor("wfcT", [128, NKH, VS], BF16, kind="ExternalInput").ap()
    whhT = nc.dram_tensor("whhT", [128, NKH, 512], BF16, kind="ExternalInput").ap()
    wihT = nc.dram_tensor("wihT", [128, NKE, 512], BF16, kind="ExternalInput").ap()
    xT = nc.dram_tensor("xT", [128, NKE, B], BF16, kind="ExternalInput").ap()
    bfc = nc.dram_tensor("bfc", [1, VS], BF16, kind="ExternalInput").ap()
    bias = nc.dram_tensor("bias", [128, 4], FP32, kind="ExternalInput").ap()
    ident = nc.dram_tensor("ident", [128, 128], BF16, kind="ExternalInput").ap()
    out = nc.dram_tensor("out", [NP, 128, VS], BF16, kind="ExternalOutput").ap()

    with tile.TileContext(nc) as tc:
        _emit(nc, tc, T, reps, wfcT, whhT, wihT, xT, bfc, bias, ident, out)
    nc.compile()
    nc.m = get_hw_module(nc.m)
    return nc


_NC_CACHE = {}


def get_nc(T, reps=1):
    key = (T, reps)
    if key not in _NC_CACHE:
        _NC_CACHE[key] = build(T, reps)
    return _NC_CACHE[key]


def _pk_chunks(mat_kT):
    """[K*128, N] k-dim-major -> [128, K, N] (k p) interleave, contiguous per
    partition line so the SBUF load is one big descriptor per partition."""
    K = mat_kT.shape[0] // 128
    return np.ascontiguousarray(
        mat_kT.reshape(K, 128, mat_kT.shape[1]).transpose(1, 0, 2)
    )


def make_in_maps(features, W_ih, W_hh, b_ih, b_hh, W_fc, b_fc):
    features = np.asarray(features, np.float32)
    W_ih = np.asarray(W_ih, np.float32)
    W_hh = np.asarray(W_hh, np.float32)
    W_fc = np.asarray(W_fc, np.float32)
    b = np.asarray(b_ih, np.float32) + np.asarray(b_hh, np.float32)
    b_fc = np.asarray(b_fc, np.float32)

    xT_np = _pk_chunks(np.ascontiguousarray(features.T)).astype(BF)
    ident_np = np.eye(128, dtype=np.float32).astype(BF)
    W_fc_pad = np.zeros((VP, H), np.float32)
    W_fc_pad[:V] = W_fc
    bfc_pad = np.zeros((VP,), np.float32)
    bfc_pad[:V] = b_fc

    in_maps = []
    for d in range(R):
        # gate order (i, f, o, g); PyTorch rows are (i, f, g, o)
        gsel = np.concatenate(
            [np.arange(g * H + d * 128, g * H + (d + 1) * 128) for g in (0, 1, 3, 2)]
        )
        whhT_np = _pk_chunks(np.ascontiguousarray(W_hh[gsel].T)).astype(BF)
        wihT_np = _pk_chunks(np.ascontiguousarray(W_ih[gsel].T)).astype(BF)
        bias_np = np.ascontiguousarray(b[gsel].reshape(4, 128).T)
        wfcT_np = _pk_chunks(
            np.ascontiguousarray(W_fc_pad[d * VS : (d + 1) * VS].T)
        ).astype(BF)
        bfc_np = bfc_pad[d * VS : (d + 1) * VS].reshape(1, VS).astype(BF)
        in_maps.append(
            {
                "wfcT": wfcT_np,
                "whhT": whhT_np,
                "wihT": wihT_np,
                "xT": xT_np,
                "bfc": bfc_np,
                "bias": bias_np,
                "ident": ident_np,
            }
        )
    return in_maps


def assemble(results, T):
    """results: list of per-core dicts with 'out' [NP, 128, VS] bf16 -> [B, T, V] fp32."""
    NP = (T + 1) // 2
    full = np.concatenate(
        [results[d]["out"].astype(np.float32) for d in range(R)], axis=2
    )  # [NP,128,VP]
    full = full.reshape(NP, 2, B, VP).transpose(2, 0, 1, 3).reshape(B, 2 * NP, VP)
    return np.ascontiguousarray(full[:, :T, :V])


def kernel(features, W_ih, W_hh, b_ih, b_hh, W_fc, b_fc, max_seq_len):
    T = int(max_seq_len)
    nc = get_nc(T)
    in_maps = make_in_maps(features, W_ih, W_hh, b_ih, b_hh, W_fc, b_fc)
    res = bass_utils.run_bass_kernel_spmd(nc, in_maps, core_ids=list(range(R)))
    return assemble(res.results, T)
